# revision 5
# baseline (speedup 1.0000x reference)
"""FAGCN (4-layer FAConv + lin1/lin2 + log_softmax) on 8 Trainium2 cores.

Strategy (graph/data parallel, per the sharding hint):
- Nodes sharded across 8 cores (6250 each). Within a core, nodes are
  degree-sorted and packed into 49 tiles of 128 (CSR layout: partition p of
  tile t = that tile's p-th node; its incoming edges occupy slot columns).
- Per layer: h-table rows [h(64) | al | pad] (128 f32 = 512B) are all-gathered
  to every core; h[src]+al[src] per edge is fetched with dma_gather.
  dma_gather indices are int16 (<32768) so the 50176-row table is covered by
  two windows: A=[0,32768) and B=[RF-32768,RF). Each node's edge list is
  split between the windows; rows in the overlap [RF-32768,32768) can use
  either window and are assigned to balance the split (no negative/skip
  indices needed -> single_packet=False large gathers are safe).
- coeff = tanh(al_src + ar_dst) * norm with ar as a per-partition ACT bias,
  messages scaled on DVE (bf16) and segment-summed via identity matmuls into
  PSUM. h_new = segsum + EPS*raw. Final: logits + log_softmax on-device.
"""
import numpy as np
from dataclasses import dataclass

import concourse.bass as bass
import concourse.bacc as bacc
import concourse.tile as tile
import concourse.mybir as mybir
from concourse import bass_utils
from concourse.masks import make_identity

F32 = mybir.dt.float32
BF16 = mybir.dt.bfloat16
I16 = mybir.dt.int16
AF = mybir.ActivationFunctionType
OP = mybir.AluOpType


@dataclass
class Cfg:
    N: int = 50000
    E: int = 800000
    F: int = 512
    H: int = 64
    C: int = 40
    L: int = 4
    EPS: float = 0.2
    M: int = 8           # cores
    CHUNK_COLS: int = 32
    WINDOW: int = 32768  # dma_gather int16 index limit

    @property
    def NSH(self):
        return self.N // self.M

    @property
    def TPC(self):
        return (self.NSH + 127) // 128

    @property
    def NSHP(self):
        return self.TPC * 128

    @property
    def RF(self):
        return self.NSHP * self.M

    @property
    def two_windows(self):
        return self.RF > self.WINDOW


def host_prep(cfg: Cfg, x, edge_index, W1, b1, W2, b2, att_l, att_r):
    """Shard + permute + build balanced window-split gather arrays."""
    N, M, NSH, NSHP, TPC = cfg.N, cfg.M, cfg.NSH, cfg.NSHP, cfg.TPC
    src = np.asarray(edge_index[0], dtype=np.int64)
    dst = np.asarray(edge_index[1], dtype=np.int64)
    loop = np.arange(N, dtype=np.int64)
    rows = np.concatenate([src, loop])
    cols = np.concatenate([dst, loop])
    deg = np.bincount(cols, minlength=N).astype(np.float32)
    dinv = (1.0 / np.sqrt(deg)).astype(np.float32)
    norm_e = (dinv[rows] * dinv[cols]).astype(np.float32)

    core_of = cols // NSH
    orders, inv_orders = [], []
    for k in range(M):
        degl = np.bincount(cols[core_of == k] - k * NSH, minlength=NSH)
        order = np.argsort(-degl, kind="stable")
        inv = np.empty(NSH, dtype=np.int64)
        inv[order] = np.arange(NSH)
        orders.append(order)
        inv_orders.append(inv)
    grow_map = np.empty(N, dtype=np.int64)
    for k in range(M):
        grow_map[k * NSH:(k + 1) * NSH] = k * NSHP + inv_orders[k]

    B_BASE = cfg.RF - cfg.WINDOW  # window B covers [B_BASE, RF)

    # pass 1: per-core per-node A/B counts -> shared CA/CB per tile
    percore = []
    CA = np.zeros(TPC, dtype=np.int64)
    CB = np.zeros(TPC, dtype=np.int64)
    for k in range(M):
        m = core_of == k
        es, en = rows[m], norm_e[m]
        rk = inv_orders[k][cols[m] - k * NSH]
        grow = grow_map[es]
        if cfg.two_windows:
            cls = np.where(grow >= cfg.WINDOW, 2,
                           np.where(grow >= B_BASE, 1, 0)).astype(np.int8)
        else:
            cls = np.zeros(len(es), np.int8)
        n0 = np.bincount(rk[cls == 0], minlength=NSHP)
        n1 = np.bincount(rk[cls == 1], minlength=NSHP)
        n2 = np.bincount(rk[cls == 2], minlength=NSHP)
        d = n0 + n1 + n2
        tgt = (d + 1) // 2
        nlo = np.minimum(np.maximum(n0, tgt), n0 + n1)
        if not cfg.two_windows:
            nlo = d
        nhi = d - nlo
        for t in range(TPC):
            s = slice(t * 128, (t + 1) * 128)
            CA[t] = max(CA[t], nlo[s].max(), 1)
            CB[t] = max(CB[t], nhi[s].max())
        percore.append((es, rk, en, grow, cls, nlo))
    offA = np.zeros(TPC + 1, dtype=np.int64)
    np.cumsum(CA, out=offA[1:])
    offB = np.zeros(TPC + 1, dtype=np.int64)
    np.cumsum(CB, out=offB[1:])
    TA, TB = int(offA[-1]), int(offB[-1])

    def wrap16(lst16):
        a = lst16.reshape(-1, 16).T.copy()
        return np.tile(a, (8, 1)).astype(np.int16)

    in_maps = []
    for k in range(M):
        es, rk, en, grow, cls, nlo = percore[k]
        # order edges per node by class (lo-fixed, flex, hi-fixed)
        o = np.lexsort((cls, rk))
        rk, en, grow, cls = rk[o], en[o], grow[o], cls[o]
        dl = np.bincount(rk, minlength=NSHP)
        run0 = np.repeat(np.cumsum(np.concatenate([[0], dl]))[:-1], dl)
        j = np.arange(len(rk)) - run0           # index within node's list
        is_lo = j < nlo[rk]
        p_all = rk % 128
        t_all = rk // 128
        colA = offA[t_all] + j                  # for lo edges
        colB = offB[t_all] + (j - nlo[rk])      # for hi edges
        posA = colA[is_lo] * 128 + p_all[is_lo]
        posB = colB[~is_lo] * 128 + p_all[~is_lo]

        idxA = np.zeros(TA * 128, dtype=np.int64)
        idxA[posA] = grow[is_lo]
        normv = np.zeros((128, TA + TB), dtype=np.float32)
        normv[p_all[is_lo], colA[is_lo]] = en[is_lo]
        if TB > 0:
            idxB = np.zeros(TB * 128, dtype=np.int64)
            idxB[posB] = grow[~is_lo] - B_BASE
            normv[p_all[~is_lo], TA + colB[~is_lo]] = en[~is_lo]
            assert idxB.min() >= 0 and idxB.max() < cfg.WINDOW
        assert idxA.max() < cfg.WINDOW

        xk = np.zeros((cfg.F, NSHP), dtype=np.float32)
        xk[:, :NSH] = np.asarray(x[k * NSH:(k + 1) * NSH], np.float32)[orders[k]].T

        im = {
            "xT": np.ascontiguousarray(xk),
            "W1T": np.ascontiguousarray(np.asarray(W1, np.float32).T),
            "b1": np.asarray(b1, np.float32).reshape(1, cfg.H),
            "W2T": np.ascontiguousarray(np.asarray(W2, np.float32).T),
            "b2": np.asarray(b2, np.float32).reshape(1, cfg.C),
            "attl": np.asarray(att_l, np.float32).reshape(1, -1),
            "attr": np.asarray(att_r, np.float32).reshape(1, -1),
            "idxA": wrap16(idxA.astype(np.int16)),
            "normv": normv,
        }
        if TB > 0:
            im["idxB"] = wrap16(idxB.astype(np.int16))
        in_maps.append(im)
    return in_maps, orders, (CA.tolist(), CB.tolist())


def build_nc(cfg: Cfg, CACB):
    CA, CB = (np.asarray(v, dtype=np.int64) for v in CACB)
    TPC, H, C, L = cfg.TPC, cfg.H, cfg.C, cfg.L
    offA = np.zeros(TPC + 1, dtype=np.int64)
    np.cumsum(CA, out=offA[1:])
    offB = np.zeros(TPC + 1, dtype=np.int64)
    np.cumsum(CB, out=offB[1:])
    TA, TB = int(offA[-1]), int(offB[-1])
    NSLC = cfg.F // 128

    nc = bacc.Bacc("TRN2", target_bir_lowering=False, debug=False,
                   num_devices=cfg.M)
    xT_h = nc.dram_tensor("xT", [cfg.F, cfg.NSHP], F32, kind="ExternalInput")
    W1T_h = nc.dram_tensor("W1T", [cfg.F, H], F32, kind="ExternalInput")
    b1_h = nc.dram_tensor("b1", [1, H], F32, kind="ExternalInput")
    W2T_h = nc.dram_tensor("W2T", [H, C], F32, kind="ExternalInput")
    b2_h = nc.dram_tensor("b2", [1, C], F32, kind="ExternalInput")
    attl_h = nc.dram_tensor("attl", [1, L * H], F32, kind="ExternalInput")
    attr_h = nc.dram_tensor("attr", [1, L * H], F32, kind="ExternalInput")
    idxA_h = nc.dram_tensor("idxA", [128, 8 * TA], I16, kind="ExternalInput")
    if TB > 0:
        idxB_h = nc.dram_tensor("idxB", [128, 8 * TB], I16, kind="ExternalInput")
    normv_h = nc.dram_tensor("normv", [128, TA + TB], F32, kind="ExternalInput")
    out_h = nc.dram_tensor("out", [cfg.NSHP, C], F32, kind="ExternalOutput")

    # chunks: consecutive tiles with both window spans <= CHUNK_COLS
    chunks = []  # (t0, t1)
    t0 = 0
    for t in range(TPC + 1):
        if t == TPC or (t > t0 and
                        (offA[t] - offA[t0] + CA[t] > cfg.CHUNK_COLS or
                         offB[t] - offB[t0] + CB[t] > cfg.CHUNK_COLS)):
            if t > t0:
                chunks.append((t0, t))
            t0 = t

    with tile.TileContext(nc) as tc:
        with tc.tile_pool(name="dram", bufs=2, space="DRAM") as dram, \
             tc.tile_pool(name="pers", bufs=1) as pers, \
             tc.tile_pool(name="gpool", bufs=2) as gpool, \
             tc.tile_pool(name="cpool", bufs=3) as cpool, \
             tc.tile_pool(name="mpool", bufs=4) as mpool, \
             tc.tile_pool(name="spool", bufs=2) as spool, \
             tc.tile_pool(name="apsum", bufs=2, space="PSUM") as apsum, \
             tc.tile_pool(name="bpsum", bufs=2, space="PSUM") as bpsum:

            ones = pers.tile([1, 128], F32)
            nc.vector.memset(ones[:], 1.0)
            ident = pers.tile([128, 128], F32)
            make_identity(nc, ident[:])
            identb = pers.tile([128, 128], BF16)
            nc.vector.tensor_copy(identb[:], ident[:])
            b1s = pers.tile([1, H], F32)
            nc.sync.dma_start(b1s[:], b1_h[:])
            b2s = pers.tile([1, C], F32)
            nc.sync.dma_start(b2s[:], b2_h[:])
            W2Ts = pers.tile([H, C], F32)
            nc.sync.dma_start(W2Ts[:], W2T_h[:])
            W1Ts = pers.tile([128, NSLC, H], F32)
            nc.sync.dma_start(W1Ts[:], W1T_h[:].rearrange("(s p) h -> p s h", p=128))
            attls = pers.tile([1, L * H], F32)
            nc.sync.dma_start(attls[:], attl_h[:])
            attrs = pers.tile([1, L * H], F32)
            nc.sync.dma_start(attrs[:], attr_h[:])
            idxA = pers.tile([128, 8 * TA], I16)
            nc.sync.dma_start(idxA[:], idxA_h[:])
            if TB > 0:
                idxB = pers.tile([128, 8 * TB], I16)
                nc.sync.dma_start(idxB[:], idxB_h[:])
            normv = pers.tile([128, TA + TB], F32)
            nc.sync.dma_start(normv[:], normv_h[:])

            attbc = pers.tile([128, max(2 * L, 1), H], F32)
            for l in range(L):
                for j, srcrow in enumerate((attls, attrs)):
                    bc = bpsum.tile([128, H], F32, tag="bc")
                    nc.tensor.matmul(bc[:], lhsT=ones[:],
                                     rhs=srcrow[0:1, l * H:(l + 1) * H],
                                     start=True, stop=True)
                    nc.vector.tensor_copy(attbc[:, 2 * l + j, :], bc[:])

            stage = pers.tile([128, TPC, 128], F32)
            nc.vector.memset(stage[:, :, H + 1:], 0.0)
            raw = pers.tile([128, TPC, H], F32)
            ar_sb = pers.tile([128, TPC], F32)
            outs = pers.tile([128, TPC, C], F32)
            mx_all = pers.tile([128, TPC], F32)
            se_all = pers.tile([128, TPC], F32)
            lse_all = pers.tile([128, TPC], F32)

            # ---- phase 0: h0 = relu(x @ W1.T + b1); al0/ar0
            XG = 4
            with nc.named_scope("p0"), tc.tile_pool(name="xpool", bufs=2) as xpool:
                for g0 in range(0, TPC, XG):
                    g1 = min(g0 + XG, TPC)
                    cw = (g1 - g0) * 128
                    xt = xpool.tile([128, NSLC, cw], F32, tag="xt")
                    nc.sync.dma_start(
                        xt[:, :, :],
                        xT_h[:, g0 * 128:g1 * 128].rearrange("(s p) c -> p s c", p=128))
                    for t in range(g0, g1):
                        lc = (t - g0) * 128
                        acc = apsum.tile([128, H], F32, tag="acc")
                        nc.tensor.matmul(acc[:], lhsT=ones[:], rhs=b1s[:],
                                         start=True, stop=False)
                        for s in range(NSLC):
                            nc.tensor.matmul(acc[:], lhsT=xt[:, s, lc:lc + 128],
                                             rhs=W1Ts[:, s, :],
                                             start=False, stop=(s == NSLC - 1))
                        nc.scalar.activation(stage[:, t, 0:H], acc[:], AF.Relu)
                        nc.vector.tensor_copy(raw[:, t, :], stage[:, t, 0:H])
                        if L > 0:
                            scr = cpool.tile([128, H], F32, tag="scr")
                            nc.vector.scalar_tensor_tensor(
                                out=scr[:], in0=stage[:, t, 0:H], scalar=1.0,
                                in1=attbc[:, 0, :], op0=OP.mult, op1=OP.mult,
                                accum_out=stage[:, t, H:H + 1])
                            scr2 = cpool.tile([128, H], F32, tag="scr")
                            nc.vector.scalar_tensor_tensor(
                                out=scr2[:], in0=stage[:, t, 0:H], scalar=1.0,
                                in1=attbc[:, 1, :], op0=OP.mult, op1=OP.mult,
                                accum_out=ar_sb[:, t:t + 1])

            # ---- layers
            for l in range(L):
                with nc.named_scope(f"ag{l}"):
                    tbl_in = dram.tile([cfg.NSHP, 128], F32, tag="tbl_in")
                    tbl_full = dram.tile([cfg.RF, 128], F32, tag="tbl_full",
                                         addr_space="Shared")
                    nc.sync.dma_start(
                        tbl_in[:].rearrange("(t p) e -> p t e", p=128),
                        stage[:])
                    nc.gpsimd.collective_compute(
                        "AllGather", OP.bypass,
                        replica_groups=[list(range(cfg.M))],
                        ins=[tbl_in.opt()], outs=[tbl_full.opt()])

                _sid, _ = nc.enter_named_scope(f"ly{l}", False)
                for (ct0, ct1) in chunks:
                    cA0, cA1 = int(offA[ct0]), int(offA[ct1])
                    cB0, cB1 = int(offB[ct0]), int(offB[ct1])
                    gA = gpool.tile([128, cfg.CHUNK_COLS, 128], F32, tag="gA")
                    nc.gpsimd.dma_gather(
                        out_ap=gA[:, :cA1 - cA0, :],
                        in_ap=(tbl_full[:cfg.WINDOW, :] if cfg.two_windows
                               else tbl_full[:, :]),
                        idxs_ap=idxA[:, 8 * cA0:8 * cA1],
                        num_idxs=128 * (cA1 - cA0),
                        num_idxs_reg=128 * (cA1 - cA0),
                        elem_size=128, single_packet=False)
                    if TB > 0 and cB1 > cB0:
                        gB = gpool.tile([128, cfg.CHUNK_COLS, 128], F32, tag="gB")
                        nc.gpsimd.dma_gather(
                            out_ap=gB[:, :cB1 - cB0, :],
                            in_ap=tbl_full[cfg.RF - cfg.WINDOW:, :],
                            idxs_ap=idxB[:, 8 * cB0:8 * cB1],
                            num_idxs=128 * (cB1 - cB0),
                            num_idxs_reg=128 * (cB1 - cB0),
                            elem_size=128, single_packet=False)
                    for t in range(ct0, ct1):
                        nA, nB = int(CA[t]), int(CB[t])
                        lcA = int(offA[t]) - cA0
                        lcB = int(offB[t]) - cB0
                        parts = [(gA, lcA, nA, int(offA[t]))]
                        if nB > 0:
                            parts.append((gB, lcB, nB, TA + int(offB[t])))
                        coeffs = []
                        for (gg, lc, nn, noff) in parts:
                            cf = cpool.tile([128, cfg.CHUNK_COLS], F32, tag="coeff")
                            nc.scalar.activation(cf[:, :nn], gg[:, lc:lc + nn, H],
                                                 AF.Tanh, bias=ar_sb[:, t:t + 1])
                            nc.vector.tensor_tensor(
                                out=cf[:, :nn], in0=cf[:, :nn],
                                in1=normv[:, noff:noff + nn], op=OP.mult)
                            coeffs.append(cf)
                        acc = apsum.tile([128, H], F32, tag="acc")
                        nblk = nA + nB
                        bi = 0
                        for (gg, lc, nn, _), cf in zip(parts, coeffs):
                            for b in range(nn):
                                msg = mpool.tile([128, H], BF16, tag="msg")
                                nc.vector.tensor_scalar(
                                    out=msg[:], in0=gg[:, lc + b, 0:H],
                                    scalar1=cf[:, b:b + 1], scalar2=None,
                                    op0=OP.mult)
                                nc.tensor.matmul(acc[:], lhsT=identb[:], rhs=msg[:],
                                                 start=(bi == 0),
                                                 stop=(bi == nblk - 1))
                                bi += 1
                        nc.vector.scalar_tensor_tensor(
                            out=stage[:, t, 0:H], in0=raw[:, t, :], scalar=cfg.EPS,
                            in1=acc[:], op0=OP.mult, op1=OP.add)
                        if l < L - 1:
                            scr = cpool.tile([128, H], F32, tag="scr")
                            nc.vector.scalar_tensor_tensor(
                                out=scr[:], in0=stage[:, t, 0:H], scalar=1.0,
                                in1=attbc[:, 2 * (l + 1), :], op0=OP.mult,
                                op1=OP.mult, accum_out=stage[:, t, H:H + 1])
                            scr2 = cpool.tile([128, H], F32, tag="scr")
                            nc.vector.scalar_tensor_tensor(
                                out=scr2[:], in0=stage[:, t, 0:H], scalar=1.0,
                                in1=attbc[:, 2 * (l + 1) + 1, :], op0=OP.mult,
                                op1=OP.mult, accum_out=ar_sb[:, t:t + 1])
                nc.leave_named_scope(f"ly{l}", _sid, False)

            # ---- logits + log_softmax
            for t in range(TPC):
                tr = bpsum.tile([H, 128], F32, tag="tr")
                nc.tensor.transpose(out=tr[:], in_=stage[:, t, 0:H],
                                    identity=ident[:])
                htT = spool.tile([H, 128], F32, tag="htT")
                nc.vector.tensor_copy(htT[:], tr[:])
                lg = bpsum.tile([128, C], F32, tag="lg")
                nc.tensor.matmul(lg[:], lhsT=ones[:], rhs=b2s[:],
                                 start=True, stop=False)
                nc.tensor.matmul(lg[:], lhsT=htT[:], rhs=W2Ts[:],
                                 start=False, stop=True)
                nc.vector.tensor_reduce(out=mx_all[:, t:t + 1], in_=lg[:],
                                        axis=mybir.AxisListType.X, op=OP.max,
                                        negate=True)
                scr40 = cpool.tile([128, C], F32, tag="scr40")
                nc.scalar.activation(scr40[:], lg[:], AF.Exp,
                                     bias=mx_all[:, t:t + 1],
                                     accum_out=se_all[:, t:t + 1])
                nc.vector.tensor_copy(outs[:, t, :], lg[:])
            nc.scalar.activation(lse_all[:], se_all[:], AF.Ln)
            for t in range(TPC):
                nc.vector.tensor_scalar(
                    out=outs[:, t, :], in0=outs[:, t, :],
                    scalar1=mx_all[:, t:t + 1], scalar2=lse_all[:, t:t + 1],
                    op0=OP.add, op1=OP.subtract)
            nc.sync.dma_start(out_h[:].rearrange("(t p) c -> p t c", p=128),
                              outs[:])
    nc.compile()
    return nc


def run(cfg: Cfg, inputs: dict, trace: bool = False):
    in_maps, orders, CACB = host_prep(cfg, **inputs)
    nc = build_nc(cfg, CACB)
    res = bass_utils.run_bass_kernel_spmd(
        nc, in_maps, core_ids=list(range(cfg.M)), trace=trace)
    out = np.empty((cfg.N, cfg.C), dtype=np.float32)
    for k in range(cfg.M):
        out[k * cfg.NSH + orders[k]] = np.asarray(res.results[k]["out"],
                                                  np.float32)[:cfg.NSH]
    return out, res


def kernel(x, edge_index, W1, b1, W2, b2, att_l, att_r):
    cfg = Cfg()
    out, _ = run(cfg, dict(x=np.asarray(x, np.float32),
                           edge_index=np.asarray(edge_index),
                           W1=W1, b1=b1, W2=W2, b2=b2,
                           att_l=att_l, att_r=att_r))
    return out



# revision 9
# speedup vs baseline: 1.4341x; 1.4341x over previous
"""FAGCN (4-layer FAConv + lin1/lin2 + log_softmax) on 8 Trainium2 cores.

Strategy (graph/data parallel):
- Nodes sharded across 8 cores (6250 each). Within a core, nodes are
  degree-sorted and packed into 49 tiles of 128 (CSR layout: partition p of
  tile t = that tile's p-th node; its incoming edges occupy slot columns).
- Per layer: h-table rows [h(64)|al|pad] (128 bf16 = 256B) are all-gathered
  to every core; h[src]+al[src] per edge is fetched with dma_gather.
  dma_gather indices are int16 (<32768) so the 50176-row table is covered by
  two windows: A=[0,32768) and B=[RF-32768,RF). Each node's edge list is
  split between the windows; rows in the overlap can use either window and
  are assigned to balance the split.
- v2 perf changes vs baseline:
  * table in bf16 (halves gather + AllGather bytes)
  * gathers round-robined over 4 SWDGE queues (parallel DMA drain)
  * per-chunk batched message scaling on DVE via stride-0 broadcast APs
    (1 op per window-part instead of 1 op per slot column)
  * segment-sum via 512-wide identity matmuls into a [128,512] PSUM tile
    (8 slots per matmul) + single strided tensor_reduce fold
  * phase-0 x@W1 in bf16
"""
import numpy as np
from dataclasses import dataclass

import ml_dtypes

import concourse.bass as bass
import concourse.bacc as bacc
import concourse.tile as tile
import concourse.mybir as mybir
from concourse import bass_utils
from concourse.masks import make_identity

F32 = mybir.dt.float32
BF16 = mybir.dt.bfloat16
I16 = mybir.dt.int16
AF = mybir.ActivationFunctionType
OP = mybir.AluOpType
NPBF16 = ml_dtypes.bfloat16


@dataclass
class Cfg:
    N: int = 50000
    E: int = 800000
    F: int = 512
    H: int = 64
    C: int = 40
    L: int = 4
    EPS: float = 0.2
    M: int = 8           # cores
    CHUNK_COLS: int = 32
    WINDOW: int = 32768  # dma_gather int16 index limit
    NQ: int = 4          # SWDGE queues for gathers

    @property
    def NSH(self):
        return self.N // self.M

    @property
    def TPC(self):
        return (self.NSH + 127) // 128

    @property
    def NSHP(self):
        return self.TPC * 128

    @property
    def RF(self):
        return self.NSHP * self.M

    @property
    def two_windows(self):
        return self.RF > self.WINDOW


def host_prep(cfg: Cfg, x, edge_index, W1, b1, W2, b2, att_l, att_r):
    """Shard + permute + build balanced window-split gather arrays."""
    N, M, NSH, NSHP, TPC = cfg.N, cfg.M, cfg.NSH, cfg.NSHP, cfg.TPC
    src = np.asarray(edge_index[0], dtype=np.int64)
    dst = np.asarray(edge_index[1], dtype=np.int64)
    loop = np.arange(N, dtype=np.int64)
    rows = np.concatenate([src, loop])
    cols = np.concatenate([dst, loop])
    deg = np.bincount(cols, minlength=N).astype(np.float32)
    dinv = (1.0 / np.sqrt(deg)).astype(np.float32)
    norm_e = (dinv[rows] * dinv[cols]).astype(np.float32)

    core_of = cols // NSH
    orders, inv_orders = [], []
    for k in range(M):
        degl = np.bincount(cols[core_of == k] - k * NSH, minlength=NSH)
        order = np.argsort(-degl, kind="stable")
        inv = np.empty(NSH, dtype=np.int64)
        inv[order] = np.arange(NSH)
        orders.append(order)
        inv_orders.append(inv)
    grow_map = np.empty(N, dtype=np.int64)
    for k in range(M):
        grow_map[k * NSH:(k + 1) * NSH] = k * NSHP + inv_orders[k]

    B_BASE = cfg.RF - cfg.WINDOW  # window B covers [B_BASE, RF)

    # pass 1: per-core per-node A/B counts -> shared CA/CB per tile
    percore = []
    CA = np.zeros(TPC, dtype=np.int64)
    CB = np.zeros(TPC, dtype=np.int64)
    for k in range(M):
        m = core_of == k
        es, en = rows[m], norm_e[m]
        rk = inv_orders[k][cols[m] - k * NSH]
        grow = grow_map[es]
        if cfg.two_windows:
            cls = np.where(grow >= cfg.WINDOW, 2,
                           np.where(grow >= B_BASE, 1, 0)).astype(np.int8)
        else:
            cls = np.zeros(len(es), np.int8)
        n0 = np.bincount(rk[cls == 0], minlength=NSHP)
        n1 = np.bincount(rk[cls == 1], minlength=NSHP)
        n2 = np.bincount(rk[cls == 2], minlength=NSHP)
        d = n0 + n1 + n2
        tgt = (d + 1) // 2
        nlo = np.minimum(np.maximum(n0, tgt), n0 + n1)
        if not cfg.two_windows:
            nlo = d
        nhi = d - nlo
        for t in range(TPC):
            s = slice(t * 128, (t + 1) * 128)
            CA[t] = max(CA[t], nlo[s].max(), 1)
            CB[t] = max(CB[t], nhi[s].max())
        percore.append((es, rk, en, grow, cls, nlo))
    offA = np.zeros(TPC + 1, dtype=np.int64)
    np.cumsum(CA, out=offA[1:])
    offB = np.zeros(TPC + 1, dtype=np.int64)
    np.cumsum(CB, out=offB[1:])
    TA, TB = int(offA[-1]), int(offB[-1])

    def wrap16(lst16):
        a = lst16.reshape(-1, 16).T.copy()
        return np.tile(a, (8, 1)).astype(np.int16)

    in_maps = []
    for k in range(M):
        es, rk, en, grow, cls, nlo = percore[k]
        # order edges per node by class (lo-fixed, flex, hi-fixed)
        o = np.lexsort((cls, rk))
        rk, en, grow, cls = rk[o], en[o], grow[o], cls[o]
        dl = np.bincount(rk, minlength=NSHP)
        run0 = np.repeat(np.cumsum(np.concatenate([[0], dl]))[:-1], dl)
        j = np.arange(len(rk)) - run0           # index within node's list
        is_lo = j < nlo[rk]
        p_all = rk % 128
        t_all = rk // 128
        colA = offA[t_all] + j                  # for lo edges
        colB = offB[t_all] + (j - nlo[rk])      # for hi edges
        posA = colA[is_lo] * 128 + p_all[is_lo]
        posB = colB[~is_lo] * 128 + p_all[~is_lo]

        idxA = np.zeros(TA * 128, dtype=np.int64)
        idxA[posA] = grow[is_lo]
        normv = np.zeros((128, TA + TB), dtype=np.float32)
        normv[p_all[is_lo], colA[is_lo]] = en[is_lo]
        if TB > 0:
            idxB = np.zeros(TB * 128, dtype=np.int64)
            idxB[posB] = grow[~is_lo] - B_BASE
            normv[p_all[~is_lo], TA + colB[~is_lo]] = en[~is_lo]
            assert idxB.min() >= 0 and idxB.max() < cfg.WINDOW
        assert idxA.max() < cfg.WINDOW

        xk = np.zeros((cfg.F, NSHP), dtype=np.float32)
        xk[:, :NSH] = np.asarray(x[k * NSH:(k + 1) * NSH], np.float32)[orders[k]].T

        im = {
            "xT": np.ascontiguousarray(xk.astype(NPBF16)),
            "W1T": np.ascontiguousarray(
                np.asarray(W1, np.float32).T.astype(NPBF16)),
            "b1": np.asarray(b1, np.float32).reshape(1, cfg.H).astype(NPBF16),
            "W2T": np.ascontiguousarray(np.asarray(W2, np.float32).T),
            "b2": np.asarray(b2, np.float32).reshape(1, cfg.C),
            "attl": np.asarray(att_l, np.float32).reshape(1, -1),
            "attr": np.asarray(att_r, np.float32).reshape(1, -1),
            "idxA": wrap16(idxA.astype(np.int16)),
            "normv": normv.astype(NPBF16),
        }
        if TB > 0:
            im["idxB"] = wrap16(idxB.astype(np.int16))
        in_maps.append(im)
    return in_maps, orders, (CA.tolist(), CB.tolist())


def build_nc(cfg: Cfg, CACB):
    CA, CB = (np.asarray(v, dtype=np.int64) for v in CACB)
    TPC, H, C, L = cfg.TPC, cfg.H, cfg.C, cfg.L
    offA = np.zeros(TPC + 1, dtype=np.int64)
    np.cumsum(CA, out=offA[1:])
    offB = np.zeros(TPC + 1, dtype=np.int64)
    np.cumsum(CB, out=offB[1:])
    TA, TB = int(offA[-1]), int(offB[-1])
    NSLC = cfg.F // 128

    nc = bacc.Bacc("TRN2", target_bir_lowering=False, debug=False,
                   num_devices=cfg.M, num_swdge_queues=cfg.NQ)
    xT_h = nc.dram_tensor("xT", [cfg.F, cfg.NSHP], BF16, kind="ExternalInput")
    W1T_h = nc.dram_tensor("W1T", [cfg.F, H], BF16, kind="ExternalInput")
    b1_h = nc.dram_tensor("b1", [1, H], BF16, kind="ExternalInput")
    W2T_h = nc.dram_tensor("W2T", [H, C], F32, kind="ExternalInput")
    b2_h = nc.dram_tensor("b2", [1, C], F32, kind="ExternalInput")
    attl_h = nc.dram_tensor("attl", [1, L * H], F32, kind="ExternalInput")
    attr_h = nc.dram_tensor("attr", [1, L * H], F32, kind="ExternalInput")
    idxA_h = nc.dram_tensor("idxA", [128, 8 * TA], I16, kind="ExternalInput")
    if TB > 0:
        idxB_h = nc.dram_tensor("idxB", [128, 8 * TB], I16, kind="ExternalInput")
    normv_h = nc.dram_tensor("normv", [128, TA + TB], BF16, kind="ExternalInput")
    out_h = nc.dram_tensor("out", [cfg.NSHP, C], F32, kind="ExternalOutput")

    # chunks: consecutive tiles with both window spans <= CHUNK_COLS
    chunks = []  # (t0, t1)
    t0 = 0
    for t in range(TPC + 1):
        if t == TPC or (t > t0 and
                        (offA[t] - offA[t0] + CA[t] > cfg.CHUNK_COLS or
                         offB[t] - offB[t0] + CB[t] > cfg.CHUNK_COLS)):
            if t > t0:
                chunks.append((t0, t))
            t0 = t

    with tile.TileContext(nc) as tc:
        with tc.tile_pool(name="dram", bufs=2, space="DRAM") as dram, \
             tc.tile_pool(name="pers", bufs=1) as pers, \
             tc.tile_pool(name="gapool", bufs=4) as gapool, \
             tc.tile_pool(name="gbpool", bufs=3) as gbpool, \
             tc.tile_pool(name="cpool", bufs=3) as cpool, \
             tc.tile_pool(name="mpool", bufs=3) as mpool, \
             tc.tile_pool(name="spool", bufs=2) as spool, \
             tc.tile_pool(name="apsum", bufs=3, space="PSUM") as apsum:

            ones = pers.tile([1, 128], F32)
            nc.vector.memset(ones[:], 1.0)
            onesb = pers.tile([1, 128], BF16)
            nc.vector.memset(onesb[:], 1.0)
            ident = pers.tile([128, 128], F32)
            make_identity(nc, ident[:])
            identb = pers.tile([128, 128], BF16)
            nc.vector.tensor_copy(identb[:], ident[:])
            b1s = pers.tile([1, H], BF16)
            nc.sync.dma_start(b1s[:], b1_h[:])
            b2s = pers.tile([1, C], F32)
            nc.sync.dma_start(b2s[:], b2_h[:])
            W2Ts = pers.tile([H, C], F32)
            nc.sync.dma_start(W2Ts[:], W2T_h[:])
            W1Ts = pers.tile([128, NSLC, H], BF16)
            nc.sync.dma_start(W1Ts[:], W1T_h[:].rearrange("(s p) h -> p s h", p=128))
            attls = pers.tile([1, L * H], F32)
            nc.sync.dma_start(attls[:], attl_h[:])
            attrs = pers.tile([1, L * H], F32)
            nc.sync.dma_start(attrs[:], attr_h[:])
            idxA = pers.tile([128, 8 * TA], I16)
            nc.sync.dma_start(idxA[:], idxA_h[:])
            if TB > 0:
                idxB = pers.tile([128, 8 * TB], I16)
                nc.sync.dma_start(idxB[:], idxB_h[:])
            normv = pers.tile([128, TA + TB], BF16)
            nc.sync.dma_start(normv[:], normv_h[:])

            attbc = pers.tile([128, max(2 * L, 1), H], F32)
            with tc.tile_pool(name="ppsum", bufs=2, space="PSUM") as ppsum:
                for l in range(L):
                    for j, srcrow in enumerate((attls, attrs)):
                        bc = ppsum.tile([128, H], F32, tag="bc")
                        nc.tensor.matmul(bc[:], lhsT=ones[:],
                                         rhs=srcrow[0:1, l * H:(l + 1) * H],
                                         start=True, stop=True)
                        nc.vector.tensor_copy(attbc[:, 2 * l + j, :], bc[:])

            # f32 h state + bf16 gather table staging
            stage = pers.tile([128, TPC, H], F32)
            tblstage = pers.tile([128, TPC, 128], BF16)
            nc.vector.memset(tblstage[:, :, H + 1:], 0.0)
            raw = pers.tile([128, TPC, H], F32)
            al_sb = pers.tile([128, TPC], F32)
            ar_sb = pers.tile([128, TPC], F32)
            outs = pers.tile([128, TPC, C], F32)
            mx_all = pers.tile([128, TPC], F32)
            se_all = pers.tile([128, TPC], F32)
            lse_all = pers.tile([128, TPC], F32)

            def attn_epilogue(t, l):
                """al/ar for layer l and bf16 table row from stage[:, t]."""
                scr = cpool.tile([128, H], F32, tag="scr")
                nc.vector.scalar_tensor_tensor(
                    out=scr[:], in0=stage[:, t, :], scalar=1.0,
                    in1=attbc[:, 2 * l, :], op0=OP.mult, op1=OP.mult,
                    accum_out=al_sb[:, t:t + 1])
                scr2 = cpool.tile([128, H], F32, tag="scr")
                nc.vector.scalar_tensor_tensor(
                    out=scr2[:], in0=stage[:, t, :], scalar=1.0,
                    in1=attbc[:, 2 * l + 1, :], op0=OP.mult, op1=OP.mult,
                    accum_out=ar_sb[:, t:t + 1])
                nc.vector.tensor_copy(tblstage[:, t, 0:H], stage[:, t, :])
                nc.vector.tensor_copy(tblstage[:, t, H:H + 1],
                                      al_sb[:, t:t + 1])

            # ---- phase 0: h0 = relu(x @ W1.T + b1); al0/ar0
            XG = 4
            with nc.named_scope("p0"), \
                 tc.tile_pool(name="xpool", bufs=2) as xpool, \
                 tc.tile_pool(name="xpsum", bufs=2, space="PSUM") as xpsum:
                for g0 in range(0, TPC, XG):
                    g1 = min(g0 + XG, TPC)
                    cw = (g1 - g0) * 128
                    xt = xpool.tile([128, NSLC, cw], BF16, tag="xt")
                    nc.sync.dma_start(
                        xt[:, :, :],
                        xT_h[:, g0 * 128:g1 * 128].rearrange("(s p) c -> p s c", p=128))
                    for t in range(g0, g1):
                        lc = (t - g0) * 128
                        acc = xpsum.tile([128, H], F32, tag="acc")
                        nc.tensor.matmul(acc[:], lhsT=onesb[:], rhs=b1s[:],
                                         start=True, stop=False)
                        for s in range(NSLC):
                            nc.tensor.matmul(acc[:], lhsT=xt[:, s, lc:lc + 128],
                                             rhs=W1Ts[:, s, :],
                                             start=False, stop=(s == NSLC - 1))
                        nc.scalar.activation(stage[:, t, :], acc[:], AF.Relu)
                        nc.vector.tensor_copy(raw[:, t, :], stage[:, t, :])
                        if L > 0:
                            attn_epilogue(t, 0)

            # ---- layers
            qi = 0
            for l in range(L):
                with nc.named_scope(f"ag{l}"):
                    tbl_in = dram.tile([cfg.NSHP, 128], BF16, tag="tbl_in")
                    tbl_full = dram.tile([cfg.RF, 128], BF16, tag="tbl_full",
                                         addr_space="Shared")
                    nc.sync.dma_start(
                        tbl_in[:].rearrange("(t p) e -> p t e", p=128),
                        tblstage[:])
                    nc.gpsimd.collective_compute(
                        "AllGather", OP.bypass,
                        replica_groups=[list(range(cfg.M))],
                        ins=[tbl_in.opt()], outs=[tbl_full.opt()])

                _sid, _ = nc.enter_named_scope(f"ly{l}", False)
                for (ct0, ct1) in chunks:
                    cA0, cA1 = int(offA[ct0]), int(offA[ct1])
                    cB0, cB1 = int(offB[ct0]), int(offB[ct1])
                    gA = gapool.tile([128, cfg.CHUNK_COLS, 128], BF16, tag="gA")
                    nc.gpsimd.dma_gather(
                        out_ap=gA[:, :cA1 - cA0, :],
                        in_ap=(tbl_full[:cfg.WINDOW, :] if cfg.two_windows
                               else tbl_full[:, :]),
                        idxs_ap=idxA[:, 8 * cA0:8 * cA1],
                        num_idxs=128 * (cA1 - cA0),
                        num_idxs_reg=128 * (cA1 - cA0),
                        elem_size=128, single_packet=False,
                        queue_num=qi % cfg.NQ)
                    qi += 1
                    if TB > 0 and cB1 > cB0:
                        gB = gbpool.tile([128, cfg.CHUNK_COLS, 128], BF16,
                                         tag="gB")
                        nc.gpsimd.dma_gather(
                            out_ap=gB[:, :cB1 - cB0, :],
                            in_ap=tbl_full[cfg.RF - cfg.WINDOW:, :],
                            idxs_ap=idxB[:, 8 * cB0:8 * cB1],
                            num_idxs=128 * (cB1 - cB0),
                            num_idxs_reg=128 * (cB1 - cB0),
                            elem_size=128, single_packet=False,
                            queue_num=qi % cfg.NQ)
                        qi += 1
                    for t in range(ct0, ct1):
                        nA, nB = int(CA[t]), int(CB[t])
                        lcA = int(offA[t]) - cA0
                        lcB = int(offB[t]) - cB0
                        parts = [(gA, lcA, nA, int(offA[t]), "A")]
                        if nB > 0:
                            parts.append((gB, lcB, nB, TA + int(offB[t]), "B"))
                        # per part: coeff = tanh(al_src + ar_dst) * norm, then
                        # one batched msg-scale op (bcast coeff along H)
                        groups = []  # (msgs_tile, slot0, width)
                        for (gg, lc, nn, noff, tag) in parts:
                            cf = cpool.tile([128, cfg.CHUNK_COLS], BF16,
                                            tag="cf" + tag)
                            nc.scalar.activation(cf[:, :nn], gg[:, lc:lc + nn, H],
                                                 AF.Tanh, bias=ar_sb[:, t:t + 1])
                            nc.vector.tensor_tensor(
                                out=cf[:, :nn], in0=cf[:, :nn],
                                in1=normv[:, noff:noff + nn], op=OP.mult)
                            msgs = mpool.tile([128, cfg.CHUNK_COLS, H], BF16,
                                              tag="msg" + tag)
                            nc.vector.tensor_tensor(
                                out=msgs[:, :nn, :],
                                in0=gg[:, lc:lc + nn, 0:H],
                                in1=cf[:, :nn].to_broadcast([128, nn, H]),
                                op=OP.mult)
                            for s0 in range(0, nn, 8):
                                groups.append((msgs, s0, min(8, nn - s0)))
                        groups.sort(key=lambda g: -g[2])
                        acc = apsum.tile([128, 512], F32, tag="acc")
                        for i, (msgs, s0, w) in enumerate(groups):
                            nc.tensor.matmul(
                                acc[:, 0:w * H],
                                lhsT=identb[:],
                                rhs=msgs[:, s0:s0 + w, :],
                                start=(i == 0), stop=(i == len(groups) - 1))
                        m = groups[0][2]  # chunks of H written in acc
                        red = cpool.tile([128, H], F32, tag="red")
                        if m > 1:
                            nc.vector.tensor_reduce(
                                out=red[:],
                                in_=acc[:, 0:m * H].rearrange(
                                    "p (m h) -> p h m", h=H),
                                axis=mybir.AxisListType.X, op=OP.add)
                            src_red = red
                        else:
                            src_red = None
                        nc.vector.scalar_tensor_tensor(
                            out=stage[:, t, :], in0=raw[:, t, :],
                            scalar=cfg.EPS,
                            in1=(src_red[:] if src_red is not None
                                 else acc[:, 0:H]),
                            op0=OP.mult, op1=OP.add)
                        if l < L - 1:
                            attn_epilogue(t, l + 1)
                nc.leave_named_scope(f"ly{l}", _sid, False)

            # ---- logits + log_softmax
            _sid, _ = nc.enter_named_scope("fin", False)
            _fpsum_cm = tc.tile_pool(name="fpsum", bufs=2, space="PSUM")
            fpsum = _fpsum_cm.__enter__()
            for t in range(TPC):
                tr = fpsum.tile([H, 128], F32, tag="tr")
                nc.tensor.transpose(out=tr[:], in_=stage[:, t, :],
                                    identity=ident[:])
                htT = spool.tile([H, 128], F32, tag="htT")
                nc.vector.tensor_copy(htT[:], tr[:])
                lg = fpsum.tile([128, C], F32, tag="lg")
                nc.tensor.matmul(lg[:], lhsT=ones[:], rhs=b2s[:],
                                 start=True, stop=False)
                nc.tensor.matmul(lg[:], lhsT=htT[:], rhs=W2Ts[:],
                                 start=False, stop=True)
                nc.vector.tensor_reduce(out=mx_all[:, t:t + 1], in_=lg[:],
                                        axis=mybir.AxisListType.X, op=OP.max,
                                        negate=True)
                scr40 = cpool.tile([128, C], F32, tag="scr40")
                nc.scalar.activation(scr40[:], lg[:], AF.Exp,
                                     bias=mx_all[:, t:t + 1],
                                     accum_out=se_all[:, t:t + 1])
                nc.vector.tensor_copy(outs[:, t, :], lg[:])
            nc.scalar.activation(lse_all[:], se_all[:], AF.Ln)
            for t in range(TPC):
                nc.vector.tensor_scalar(
                    out=outs[:, t, :], in0=outs[:, t, :],
                    scalar1=mx_all[:, t:t + 1], scalar2=lse_all[:, t:t + 1],
                    op0=OP.add, op1=OP.subtract)
            nc.sync.dma_start(out_h[:].rearrange("(t p) c -> p t c", p=128),
                              outs[:])
            _fpsum_cm.__exit__(None, None, None)
            nc.leave_named_scope("fin", _sid, False)
    nc.compile()
    return nc


def run(cfg: Cfg, inputs: dict, trace: bool = False):
    in_maps, orders, CACB = host_prep(cfg, **inputs)
    nc = build_nc(cfg, CACB)
    res = bass_utils.run_bass_kernel_spmd(
        nc, in_maps, core_ids=list(range(cfg.M)), trace=trace)
    out = np.empty((cfg.N, cfg.C), dtype=np.float32)
    for k in range(cfg.M):
        out[k * cfg.NSH + orders[k]] = np.asarray(res.results[k]["out"],
                                                  np.float32)[:cfg.NSH]
    return out, res


def kernel(x, edge_index, W1, b1, W2, b2, att_l, att_r):
    cfg = Cfg()
    out, _ = run(cfg, dict(x=np.asarray(x, np.float32),
                           edge_index=np.asarray(edge_index),
                           W1=W1, b1=b1, W2=W2, b2=b2,
                           att_l=att_l, att_r=att_r))
    return out


# revision 28
# speedup vs baseline: 2.3390x; 1.6310x over previous
"""FAGCN (4-layer FAConv + lin1/lin2 + log_softmax) on 8 Trainium2 cores.

Strategy (graph/data parallel):
- Nodes sharded across 8 cores (6250 each -> 49 tiles of 128). Per-core CSR
  slot layout: partition p of tile t = that tile's p-th node; its incoming
  edges occupy slot columns; padding slots gather row 0 with norm 0.
- The per-layer h-table row is [h(64)|al|pad] (128 bf16 = 256B). Instead of
  one 50176-row table (which would exceed dma_gather's int16 index range),
  the table is split into two overlapping 32768-row tables:
    T0 = tiles [0,32) of every core, T1 = tiles [17,49) of every core.
  Each is all-gathered separately, so AG0 can be issued mid-way through the
  previous layer (as soon as tiles 0..31 are computed) and its transfer
  overlaps the remaining compute + gathers; only AG1 is issued at the layer
  boundary, and the first few T0-gathers of the next layer are prefetched
  ahead of the first T1-gather so AG1's transfer is hidden too.
- Each node's edge list is split between T0/T1; edges from flex nodes
  (tiles [17,32)) can use either side and are assigned to balance the
  split. Nodes are packed into tiles by similar (nlo,nhi) so the per-tile
  slot maxima are tight (less gather padding).
- Gathers are round-robined over 4 SWDGE queues; message scaling is one
  batched DVE op per chunk (stride-0 broadcast of the coeff); segment-sum
  via 512-wide identity matmuls into PSUM + one strided tensor_reduce.
"""
import numpy as np
from dataclasses import dataclass

import ml_dtypes

import concourse.bass as bass
import concourse.bacc as bacc
import concourse.tile as tile
import concourse.mybir as mybir
from concourse import bass_utils
from concourse.masks import make_identity

F32 = mybir.dt.float32
BF16 = mybir.dt.bfloat16
I16 = mybir.dt.int16
AF = mybir.ActivationFunctionType
OP = mybir.AluOpType
NPBF16 = ml_dtypes.bfloat16


@dataclass
class Cfg:
    N: int = 50000
    E: int = 800000
    F: int = 512
    H: int = 64
    C: int = 40
    L: int = 4
    EPS: float = 0.2
    M: int = 8           # cores
    CHUNK_COLS: int = 32
    NQ: int = 4          # SWDGE queues for gathers
    G0T: int = 32        # T0 covers tiles [0, G0T)
    G1S: int = 17        # T1 covers tiles [G1S, TPC)
    PRE: int = 5         # A-gather prefetch distance (chunks)
    PREB: int = 3        # B-gather prefetch distance (chunks)
    AGLOOK: int = 1      # chunks of lookahead before issuing mid-layer AG0

    @property
    def NSH(self):
        return self.N // self.M

    @property
    def TPC(self):
        return (self.NSH + 127) // 128

    @property
    def NSHP(self):
        return self.TPC * 128

    @property
    def T0PC(self):
        return self.G0T * 128          # per-core rows in T0

    @property
    def T1PC(self):
        return (self.TPC - self.G1S) * 128  # per-core rows in T1


def host_prep(cfg: Cfg, x, edge_index, W1, b1, W2, b2, att_l, att_r):
    """Shard + permute + build balanced two-table gather arrays."""
    N, M, NSH, NSHP, TPC = cfg.N, cfg.M, cfg.NSH, cfg.NSHP, cfg.TPC
    src = np.asarray(edge_index[0], dtype=np.int64)
    dst = np.asarray(edge_index[1], dtype=np.int64)
    deg = (np.bincount(dst, minlength=N) + 1).astype(np.float32)
    dinv = (1.0 / np.sqrt(deg)).astype(np.float32)
    rows, cols = src, dst            # self-loops handled on-chip
    norm_e = (dinv[rows] * dinv[cols]).astype(np.float32)

    core_of = cols // NSH
    FLEX0 = cfg.G1S * 128              # first flex rank
    FLEX1 = cfg.G0T * 128              # first T1-only rank

    # ---- round 1: degree order fixes each node's rank-group (0/1/2)
    cls_node = np.empty(N, dtype=np.int8)   # 0: T0-only, 1: flex, 2: T1-only
    grp_rank1 = []
    for k in range(M):
        degl = np.bincount(cols[core_of == k] - k * NSH, minlength=NSH)
        order1 = np.argsort(-degl, kind="stable")
        inv1 = np.empty(NSH, dtype=np.int64)
        inv1[order1] = np.arange(NSH)
        c = np.where(inv1 < FLEX0, 0,
                     np.where(inv1 < FLEX1, 1, 2)).astype(np.int8)
        cls_node[k * NSH:(k + 1) * NSH] = c
        grp_rank1.append(order1)

    # ---- per-core balance: per-dst-node window counts under edge classes
    percore = []
    nlo_all = np.empty(N, dtype=np.int64)
    nhi_all = np.empty(N, dtype=np.int64)
    for k in range(M):
        m = core_of == k
        es, en = rows[m], norm_e[m]
        ln = cols[m] - k * NSH
        cls_e = cls_node[es]
        n0 = np.bincount(ln[cls_e == 0], minlength=NSH)
        n1 = np.bincount(ln[cls_e == 1], minlength=NSH)
        n2 = np.bincount(ln[cls_e == 2], minlength=NSH)
        d = n0 + n1 + n2
        tgt = (d + 1) // 2
        nlo = np.minimum(np.maximum(n0, tgt), n0 + n1)
        nhi = d - nlo
        nlo_all[k * NSH:(k + 1) * NSH] = nlo
        nhi_all[k * NSH:(k + 1) * NSH] = nhi
        percore.append((es, en, ln, cls_e))

    # ---- round 2: within each rank-group, sort nodes by (nlo, nhi)
    orders, inv_orders = [], []
    for k in range(M):
        order1 = grp_rank1[k]
        nlo = nlo_all[k * NSH:(k + 1) * NSH]
        nhi = nhi_all[k * NSH:(k + 1) * NSH]
        order2 = np.empty(NSH, dtype=np.int64)
        for a, b in ((0, FLEX0), (FLEX0, FLEX1), (FLEX1, NSH)):
            b = min(b, NSH)
            if a >= b:
                continue
            grp_nodes = order1[a:b]
            o = np.lexsort((nhi[grp_nodes], nlo[grp_nodes]))
            order2[a:b] = grp_nodes[o]
        inv2 = np.empty(NSH, dtype=np.int64)
        inv2[order2] = np.arange(NSH)
        orders.append(order2)
        inv_orders.append(inv2)

    # global row maps into T0 / T1
    grow0 = np.full(N, -1, dtype=np.int64)
    grow1 = np.full(N, -1, dtype=np.int64)
    for k in range(M):
        r = inv_orders[k]
        in0 = np.nonzero(r < FLEX1)[0]
        grow0[k * NSH + in0] = k * cfg.T0PC + r[in0]
        in1 = np.nonzero(r >= FLEX0)[0]
        grow1[k * NSH + in1] = k * cfg.T1PC + (r[in1] - FLEX0)

    # ---- shared CA/CB per tile (max over cores, under order2)
    CA = np.zeros(TPC, dtype=np.int64)
    CB = np.zeros(TPC, dtype=np.int64)
    for k in range(M):
        nlo = nlo_all[k * NSH:(k + 1) * NSH][orders[k]]
        nhi = nhi_all[k * NSH:(k + 1) * NSH][orders[k]]
        nlo = np.concatenate([nlo, np.zeros(NSHP - NSH, np.int64)])
        nhi = np.concatenate([nhi, np.zeros(NSHP - NSH, np.int64)])
        for t in range(TPC):
            s = slice(t * 128, (t + 1) * 128)
            CA[t] = max(CA[t], nlo[s].max(), 1)
            CB[t] = max(CB[t], nhi[s].max())
    offA = np.zeros(TPC + 1, dtype=np.int64)
    np.cumsum(CA, out=offA[1:])
    offB = np.zeros(TPC + 1, dtype=np.int64)
    np.cumsum(CB, out=offB[1:])
    TA, TB = int(offA[-1]), int(offB[-1])

    def wrap16(lst16):
        a = lst16.reshape(-1, 16).T.copy()
        return np.tile(a, (8, 1)).astype(np.int16)

    in_maps = []
    for k in range(M):
        es, en, ln, cls_e = percore[k]
        rk = inv_orders[k][ln]
        nlo_rk = nlo_all[k * NSH + ln]          # per-edge: its dst node's nlo
        # order edges per node by class (T0-fixed, flex, T1-fixed)
        o = np.lexsort((cls_e, rk))
        rk, en2, es2 = rk[o], en[o], es[o]
        nlo_rk = nlo_rk[o]
        dl = np.bincount(rk, minlength=NSHP)
        run0 = np.repeat(np.cumsum(np.concatenate([[0], dl]))[:-1], dl)
        j = np.arange(len(rk)) - run0           # index within node's list
        is_lo = j < nlo_rk
        p_all = rk % 128
        t_all = rk // 128
        colA = offA[t_all] + j
        colB = offB[t_all] + (j - nlo_rk)
        posA = colA[is_lo] * 128 + p_all[is_lo]
        posB = colB[~is_lo] * 128 + p_all[~is_lo]

        idxA = np.zeros(TA * 128, dtype=np.int64)
        vA = grow0[es2[is_lo]]
        assert vA.min() >= 0 and vA.max() < cfg.T0PC * M
        idxA[posA] = vA
        normv = np.zeros((128, TA + TB), dtype=np.float32)
        normv[p_all[is_lo], colA[is_lo]] = en2[is_lo]
        if TB > 0:
            idxB = np.zeros(TB * 128, dtype=np.int64)
            vB = grow1[es2[~is_lo]]
            assert vB.min() >= 0 and vB.max() < cfg.T1PC * M
            idxB[posB] = vB
            normv[p_all[~is_lo], TA + colB[~is_lo]] = en2[~is_lo]

        xk = np.zeros((cfg.F, NSHP), dtype=np.float32)
        xk[:, :NSH] = np.asarray(x[k * NSH:(k + 1) * NSH], np.float32)[orders[k]].T
        d2 = np.zeros(NSHP, dtype=np.float32)
        d2[:NSH] = (dinv[k * NSH:(k + 1) * NSH] ** 2)[orders[k]]
        d2 = d2.reshape(TPC, 128).T.copy()      # [128, TPC]

        im = {
            "xT": np.ascontiguousarray(xk.astype(NPBF16)),
            "W1T": np.ascontiguousarray(
                np.asarray(W1, np.float32).T.astype(NPBF16)),
            "b1": np.asarray(b1, np.float32).reshape(1, cfg.H).astype(NPBF16),
            "W2T": np.ascontiguousarray(np.asarray(W2, np.float32).T),
            "b2": np.asarray(b2, np.float32).reshape(1, cfg.C),
            "attl": np.asarray(att_l, np.float32).reshape(1, -1),
            "attr": np.asarray(att_r, np.float32).reshape(1, -1),
            "idxA": wrap16(idxA.astype(np.int16)),
            "normv": normv.astype(NPBF16),
            "dinv2": d2,
        }
        if TB > 0:
            im["idxB"] = wrap16(idxB.astype(np.int16))
        in_maps.append(im)
    return in_maps, orders, (CA.tolist(), CB.tolist())


def build_nc(cfg: Cfg, CACB):
    CA, CB = (np.asarray(v, dtype=np.int64) for v in CACB)
    TPC, H, C, L = cfg.TPC, cfg.H, cfg.C, cfg.L
    offA = np.zeros(TPC + 1, dtype=np.int64)
    np.cumsum(CA, out=offA[1:])
    offB = np.zeros(TPC + 1, dtype=np.int64)
    np.cumsum(CB, out=offB[1:])
    TA, TB = int(offA[-1]), int(offB[-1])
    NSLC = cfg.F // 128

    nc = bacc.Bacc("TRN2", target_bir_lowering=False, debug=False,
                   num_devices=cfg.M, num_swdge_queues=cfg.NQ,
                   dynamic_dma_scratch_size=16384)
    xT_h = nc.dram_tensor("xT", [cfg.F, cfg.NSHP], BF16, kind="ExternalInput")
    W1T_h = nc.dram_tensor("W1T", [cfg.F, H], BF16, kind="ExternalInput")
    b1_h = nc.dram_tensor("b1", [1, H], BF16, kind="ExternalInput")
    W2T_h = nc.dram_tensor("W2T", [H, C], F32, kind="ExternalInput")
    b2_h = nc.dram_tensor("b2", [1, C], F32, kind="ExternalInput")
    attl_h = nc.dram_tensor("attl", [1, L * H], F32, kind="ExternalInput")
    attr_h = nc.dram_tensor("attr", [1, L * H], F32, kind="ExternalInput")
    idxA_h = nc.dram_tensor("idxA", [128, 8 * TA], I16, kind="ExternalInput")
    if TB > 0:
        idxB_h = nc.dram_tensor("idxB", [128, 8 * TB], I16, kind="ExternalInput")
    normv_h = nc.dram_tensor("normv", [128, TA + TB], BF16, kind="ExternalInput")
    dinv2_h = nc.dram_tensor("dinv2", [128, TPC], F32, kind="ExternalInput")
    out_h = nc.dram_tensor("out", [cfg.NSHP, C], F32, kind="ExternalOutput")

    # chunks: consecutive tiles with both window spans <= CHUNK_COLS
    chunks = []  # (t0, t1)
    t0 = 0
    for t in range(TPC + 1):
        if t == TPC or (t > t0 and
                        (offA[t] - offA[t0] + CA[t] > cfg.CHUNK_COLS or
                         offB[t] - offB[t0] + CB[t] > cfg.CHUNK_COLS)):
            if t > t0:
                chunks.append((t0, t))
            t0 = t
    NCH = len(chunks)
    # chunk index after which all tiles < G0T have been computed (+ lookahead)
    ag0_after = next(i for i, (a, b) in enumerate(chunks) if b >= cfg.G0T)
    ag0_after = min(ag0_after + cfg.AGLOOK, NCH - 1)

    with tile.TileContext(nc) as tc:
        with tc.tile_pool(name="dram", bufs=2, space="DRAM") as dram, \
             tc.tile_pool(name="pers", bufs=1) as pers, \
             tc.tile_pool(name="gapool", bufs=7) as gapool, \
             tc.tile_pool(name="gbpool", bufs=5) as gbpool, \
             tc.tile_pool(name="cpool", bufs=3) as cpool, \
             tc.tile_pool(name="mpool", bufs=3) as mpool, \
             tc.tile_pool(name="spool", bufs=2) as spool, \
             tc.tile_pool(name="apsum", bufs=3, space="PSUM") as apsum:

            ones = pers.tile([1, 128], F32)
            nc.vector.memset(ones[:], 1.0)
            onesb = pers.tile([1, 128], BF16)
            nc.vector.memset(onesb[:], 1.0)
            ident = pers.tile([128, 128], F32)
            make_identity(nc, ident[:])
            identb = pers.tile([128, 128], BF16)
            nc.vector.tensor_copy(identb[:], ident[:])
            b1s = pers.tile([1, H], BF16)
            nc.sync.dma_start(b1s[:], b1_h[:])
            b2s = pers.tile([1, C], F32)
            nc.sync.dma_start(b2s[:], b2_h[:])
            W2Ts = pers.tile([H, C], F32)
            nc.sync.dma_start(W2Ts[:], W2T_h[:])
            W1Ts = pers.tile([128, NSLC, H], BF16)
            nc.sync.dma_start(W1Ts[:], W1T_h[:].rearrange("(s p) h -> p s h", p=128))
            attls = pers.tile([1, L * H], F32)
            nc.sync.dma_start(attls[:], attl_h[:])
            attrs = pers.tile([1, L * H], F32)
            nc.sync.dma_start(attrs[:], attr_h[:])
            idxA = pers.tile([128, 8 * TA], I16)
            nc.sync.dma_start(idxA[:], idxA_h[:])
            if TB > 0:
                idxB = pers.tile([128, 8 * TB], I16)
                nc.sync.dma_start(idxB[:], idxB_h[:])
            normv = pers.tile([128, TA + TB], BF16)
            nc.sync.dma_start(normv[:], normv_h[:])
            dinv2s = pers.tile([128, TPC], F32)
            nc.sync.dma_start(dinv2s[:], dinv2_h[:])
            cs_tmp = pers.tile([128, TPC], F32)
            cs_all = pers.tile([128, TPC], F32)

            warm_sb = pers.tile([8, 128], BF16)
            nc.vector.memset(warm_sb[:], 0.0)
            warm_in = dram.tile([8, 128], BF16, tag="warm_in")
            warm_out = dram.tile([8 * cfg.M, 128], BF16, tag="warm_out",
                                 addr_space="Shared")
            nc.sync.dma_start(warm_in[:], warm_sb[:])
            nc.gpsimd.collective_compute(
                "AllGather", OP.bypass,
                replica_groups=[list(range(cfg.M))],
                ins=[warm_in.opt()], outs=[warm_out.opt()])

            attbc = pers.tile([128, max(2 * L, 1), H], F32)
            with tc.tile_pool(name="ppsum", bufs=2, space="PSUM") as ppsum:
                for l in range(L):
                    for j, srcrow in enumerate((attls, attrs)):
                        bc = ppsum.tile([128, H], F32, tag="bc")
                        nc.tensor.matmul(bc[:], lhsT=ones[:],
                                         rhs=srcrow[0:1, l * H:(l + 1) * H],
                                         start=True, stop=True)
                        nc.vector.tensor_copy(attbc[:, 2 * l + j, :], bc[:])

            # f32 h state + bf16 gather-table staging
            stage = pers.tile([128, TPC, H], F32)
            tblstage = pers.tile([128, TPC, 128], BF16)
            nc.vector.memset(tblstage[:, :, H + 1:], 0.0)
            raw = pers.tile([128, TPC, H], F32)
            al_sb = pers.tile([128, TPC], F32)
            ar_sb = pers.tile([128, TPC], F32)
            outs = pers.tile([128, TPC, C], F32)
            mx_all = pers.tile([128, TPC], F32)
            se_all = pers.tile([128, TPC], F32)
            lse_all = pers.tile([128, TPC], F32)

            def attn_epilogue(t, l):
                """al/ar for layer l and bf16 table row from stage[:, t]."""
                scr = cpool.tile([128, H], F32, tag="scr")
                nc.vector.scalar_tensor_tensor(
                    out=scr[:], in0=stage[:, t, :], scalar=1.0,
                    in1=attbc[:, 2 * l, :], op0=OP.mult, op1=OP.mult,
                    accum_out=al_sb[:, t:t + 1])
                scr2 = cpool.tile([128, H], F32, tag="scr")
                nc.vector.scalar_tensor_tensor(
                    out=scr2[:], in0=stage[:, t, :], scalar=1.0,
                    in1=attbc[:, 2 * l + 1, :], op0=OP.mult, op1=OP.mult,
                    accum_out=ar_sb[:, t:t + 1])
                nc.scalar.activation(tblstage[:, t, 0:H], stage[:, t, :],
                                     AF.Copy)
                nc.scalar.activation(tblstage[:, t, H:H + 1],
                                     al_sb[:, t:t + 1], AF.Copy)

            def issue_ag(idx):
                """AllGather T0 (tiles [0,G0T)) or T1 (tiles [G1S,TPC))."""
                npc = cfg.T0PC if idx == 0 else cfg.T1PC
                sl = (slice(0, cfg.G0T) if idx == 0
                      else slice(cfg.G1S, TPC))
                tbl_in = dram.tile([npc, 128], BF16, tag=f"tbl_in{idx}")
                tbl_out = dram.tile([npc * cfg.M, 128], BF16,
                                    tag=f"tbl{idx}", addr_space="Shared")
                nc.sync.dma_start(
                    tbl_in[:].rearrange("(t p) e -> p t e", p=128),
                    tblstage[:, sl, :])
                nc.gpsimd.collective_compute(
                    "AllGather", OP.bypass,
                    replica_groups=[list(range(cfg.M))],
                    ins=[tbl_in.opt()], outs=[tbl_out.opt()])
                return tbl_out

            # ---- phase 0: h0 = relu(x @ W1.T + b1); al0/ar0
            XG = 4
            tbl0_next = tbl1_next = None
            with nc.named_scope("p0"), \
                 tc.tile_pool(name="xpool", bufs=2) as xpool, \
                 tc.tile_pool(name="xpsum", bufs=2, space="PSUM") as xpsum:
                for g0 in range(0, TPC, XG):
                    g1 = min(g0 + XG, TPC)
                    cw = (g1 - g0) * 128
                    xt = xpool.tile([128, NSLC, cw], BF16, tag="xt")
                    nc.sync.dma_start(
                        xt[:, :, :],
                        xT_h[:, g0 * 128:g1 * 128].rearrange("(s p) c -> p s c", p=128))
                    for t in range(g0, g1):
                        lc = (t - g0) * 128
                        acc = xpsum.tile([128, H], F32, tag="acc")
                        nc.tensor.matmul(acc[:], lhsT=onesb[:], rhs=b1s[:],
                                         start=True, stop=False)
                        for s in range(NSLC):
                            nc.tensor.matmul(acc[:], lhsT=xt[:, s, lc:lc + 128],
                                             rhs=W1Ts[:, s, :],
                                             start=False, stop=(s == NSLC - 1))
                        nc.scalar.activation(stage[:, t, :], acc[:], AF.Relu)
                        nc.vector.tensor_copy(raw[:, t, :], stage[:, t, :])
                        attn_epilogue(t, 0)
                    if g1 >= cfg.G0T and g0 < cfg.G0T:
                        tbl0_next = issue_ag(0)
                tbl1_next = issue_ag(1)

            _fpsum_cm = tc.tile_pool(name="fpsum", bufs=2, space="PSUM")
            fpsum = _fpsum_cm.__enter__()

            def finalize_tile(t):
                """logits + per-node log_softmax for tile t."""
                tr = fpsum.tile([H, 128], F32, tag="tr")
                nc.tensor.transpose(out=tr[:], in_=stage[:, t, :],
                                    identity=ident[:])
                htT = spool.tile([H, 128], F32, tag="htT")
                nc.vector.tensor_copy(htT[:], tr[:])
                lg = fpsum.tile([128, C], F32, tag="lg")
                nc.tensor.matmul(lg[:], lhsT=ones[:], rhs=b2s[:],
                                 start=True, stop=False)
                nc.tensor.matmul(lg[:], lhsT=htT[:], rhs=W2Ts[:],
                                 start=False, stop=True)
                nc.vector.tensor_reduce(out=mx_all[:, t:t + 1], in_=lg[:],
                                        axis=mybir.AxisListType.X, op=OP.max,
                                        negate=True)
                scr40 = cpool.tile([128, C], F32, tag="scr40")
                nc.scalar.activation(scr40[:], lg[:], AF.Exp,
                                     bias=mx_all[:, t:t + 1],
                                     accum_out=se_all[:, t:t + 1])
                nc.scalar.activation(lse_all[:, t:t + 1],
                                     se_all[:, t:t + 1], AF.Ln)
                nc.vector.tensor_scalar(
                    out=outs[:, t, :], in0=lg[:],
                    scalar1=mx_all[:, t:t + 1], scalar2=lse_all[:, t:t + 1],
                    op0=OP.add, op1=OP.subtract)

            # ---- layers
            qi = 0
            gA_tiles = {}
            gB_tiles = {}

            def emit_A(ci, tbl0):
                nonlocal qi
                (a0, a1) = chunks[ci]
                cA0, cA1 = int(offA[a0]), int(offA[a1])
                gA = gapool.tile([128, cfg.CHUNK_COLS, 128], BF16, tag="gA")
                nc.gpsimd.dma_gather(
                    out_ap=gA[:, :cA1 - cA0, :],
                    in_ap=tbl0[:, :],
                    idxs_ap=idxA[:, 8 * cA0:8 * cA1],
                    num_idxs=128 * (cA1 - cA0),
                    num_idxs_reg=128 * (cA1 - cA0),
                    elem_size=128, single_packet=False,
                    queue_num=qi % cfg.NQ)
                qi += 1
                gA_tiles[ci] = gA

            # prefetch layer-0 A-gathers (tbl0 for layer 0 came from p0's AG0)
            for ci in range(min(cfg.PRE, NCH)):
                emit_A(ci, tbl0_next)

            def emit_B(ci, tbl1):
                nonlocal qi
                (a0, a1) = chunks[ci]
                cB0, cB1 = int(offB[a0]), int(offB[a1])
                wB = cB1 - cB0
                if TB == 0 or wB == 0:
                    return
                gB = gbpool.tile([128, cfg.CHUNK_COLS, 128], BF16, tag="gB")
                nc.gpsimd.dma_gather(
                    out_ap=gB[:, :wB, :],
                    in_ap=tbl1[:, :],
                    idxs_ap=idxB[:, 8 * cB0:8 * cB1],
                    num_idxs=128 * wB,
                    num_idxs_reg=128 * wB,
                    elem_size=128, single_packet=False,
                    queue_num=qi % cfg.NQ)
                qi += 1
                gB_tiles[ci] = gB

            for l in range(L):
                tbl0, tbl1 = tbl0_next, tbl1_next
                tbl0_next = tbl1_next = None
                _sid, _ = nc.enter_named_scope(f"ly{l}", False)
                # self-loop coeff: cs = tanh(al + ar) * dinv^2  [128, TPC]
                nc.vector.tensor_tensor(out=cs_tmp[:], in0=al_sb[:],
                                        in1=ar_sb[:], op=OP.add)
                nc.scalar.activation(cs_all[:], cs_tmp[:], AF.Tanh)
                nc.vector.tensor_tensor(out=cs_all[:], in0=cs_all[:],
                                        in1=dinv2s[:], op=OP.mult)
                for ci in range(min(cfg.PREB, NCH)):
                    emit_B(ci, tbl1)
                for ci, (ct0, ct1) in enumerate(chunks):
                    cA0, cA1 = int(offA[ct0]), int(offA[ct1])
                    cB0, cB1 = int(offB[ct0]), int(offB[ct1])
                    wA, wB = cA1 - cA0, cB1 - cB0
                    if ci + cfg.PREB < NCH:
                        emit_B(ci + cfg.PREB, tbl1)
                    if ci + cfg.PRE < NCH:
                        emit_A(ci + cfg.PRE, tbl0)
                    gA = gA_tiles.pop(ci)
                    gB = gB_tiles.pop(ci, None)

                    # chunk-wide coeff + msgs per window part
                    parts = {}
                    for kind, gg, w, c0, noff in (
                            ("A", gA, wA, cA0, 0),
                            ("B", gB, wB, cB0, TA)):
                        if gg is None or w == 0:
                            continue
                        cf = cpool.tile([128, cfg.CHUNK_COLS], BF16,
                                        tag="cf" + kind)
                        off = offA if kind == "A" else offB
                        cnt = CA if kind == "A" else CB
                        for t in range(ct0, ct1):
                            nn = int(cnt[t])
                            if nn == 0:
                                continue
                            lc = int(off[t]) - c0
                            nc.scalar.activation(
                                cf[:, lc:lc + nn], gg[:, lc:lc + nn, H],
                                AF.Tanh, bias=ar_sb[:, t:t + 1])
                        nc.vector.tensor_tensor(
                            out=cf[:, :w], in0=cf[:, :w],
                            in1=normv[:, noff + c0:noff + c0 + w], op=OP.mult)
                        msgs = mpool.tile([128, cfg.CHUNK_COLS, H], BF16,
                                          tag="msg" + kind)
                        nc.vector.tensor_tensor(
                            out=msgs[:, :w, :],
                            in0=gg[:, :w, 0:H],
                            in1=cf[:, :w].to_broadcast([128, w, H]),
                            op=OP.mult)
                        parts[kind] = msgs

                    for t in range(ct0, ct1):
                        nA, nB = int(CA[t]), int(CB[t])
                        groups = []  # (msgs, slot0, width)
                        lcA = int(offA[t]) - cA0
                        for s0 in range(0, nA, 8):
                            groups.append((parts["A"], lcA + s0,
                                           min(8, nA - s0)))
                        if nB > 0:
                            lcB = int(offB[t]) - cB0
                            for s0 in range(0, nB, 8):
                                groups.append((parts["B"], lcB + s0,
                                               min(8, nB - s0)))
                        msgS = mpool.tile([128, 1, H], BF16, tag="msgS")
                        nc.vector.tensor_scalar(
                            out=msgS[:, 0, :], in0=stage[:, t, :],
                            scalar1=cs_all[:, t:t + 1], scalar2=None,
                            op0=OP.mult)
                        groups.append((msgS, 0, 1))
                        groups.sort(key=lambda g: -g[2])
                        acc = apsum.tile([128, 512], F32, tag="acc")
                        for i, (msgs, s0, w) in enumerate(groups):
                            nc.tensor.matmul(
                                acc[:, 0:w * H],
                                lhsT=identb[:],
                                rhs=msgs[:, s0:s0 + w, :],
                                start=(i == 0), stop=(i == len(groups) - 1))
                        m = groups[0][2]
                        red = cpool.tile([128, H], F32, tag="red")
                        if m > 1:
                            nc.vector.tensor_reduce(
                                out=red[:],
                                in_=acc[:, 0:m * H].rearrange(
                                    "p (m h) -> p h m", h=H),
                                axis=mybir.AxisListType.X, op=OP.add)
                            rsrc = red[:]
                        else:
                            rsrc = acc[:, 0:H]
                        nc.vector.scalar_tensor_tensor(
                            out=stage[:, t, :], in0=raw[:, t, :],
                            scalar=cfg.EPS, in1=rsrc,
                            op0=OP.mult, op1=OP.add)
                        if l < L - 1:
                            attn_epilogue(t, l + 1)
                        else:
                            finalize_tile(t)
                    if l < L - 1 and ci == ag0_after:
                        tbl0_next = issue_ag(0)
                if l < L - 1:
                    for ci in range(min(cfg.PRE, NCH)):
                        emit_A(ci, tbl0_next)
                    tbl1_next = issue_ag(1)
                nc.leave_named_scope(f"ly{l}", _sid, False)

            # ---- output writeback (logits computed inline in last layer)
            _sid, _ = nc.enter_named_scope("fin", False)
            nc.sync.dma_start(out_h[:].rearrange("(t p) c -> p t c", p=128),
                              outs[:])
            _fpsum_cm.__exit__(None, None, None)
            nc.leave_named_scope("fin", _sid, False)
    nc.compile()
    return nc


def run(cfg: Cfg, inputs: dict, trace: bool = False):
    in_maps, orders, CACB = host_prep(cfg, **inputs)
    nc = build_nc(cfg, CACB)
    res = bass_utils.run_bass_kernel_spmd(
        nc, in_maps, core_ids=list(range(cfg.M)), trace=trace)
    out = np.empty((cfg.N, cfg.C), dtype=np.float32)
    for k in range(cfg.M):
        out[k * cfg.NSH + orders[k]] = np.asarray(res.results[k]["out"],
                                                  np.float32)[:cfg.NSH]
    return out, res


def kernel(x, edge_index, W1, b1, W2, b2, att_l, att_r):
    cfg = Cfg()
    out, _ = run(cfg, dict(x=np.asarray(x, np.float32),
                           edge_index=np.asarray(edge_index),
                           W1=W1, b1=b1, W2=W2, b2=b2,
                           att_l=att_l, att_r=att_r))
    return out
